# revision 11
# baseline (speedup 1.0000x reference)
"""Canny-filter Trainium2 kernel: 16x[3,768,768] fp32, batch-sharded over 8 NeuronCores.

Pipeline per core (2 images, 7 H-tiles of 128 rows, stride 124 valid rows):
  - vertical 3-tap convs as banded 128x128 fp32 matmuls on TensorE
  - horizontal 3-tap passes + pointwise on VectorE/GpSimd (scalar_tensor_tensor fusion)
  - PSUM->SBUF copies, Square/Sqrt/Arctan on ScalarE
  - grad_orientation via arctan + fused round-to-nearest-even int32 cast
  - thin_edges == grad_magnitude exactly (w_dir are center-1 identity kernels)
"""
import numpy as np
from contextlib import ExitStack

import concourse.bass as bass
import concourse.bacc as bacc
import concourse.tile as tile
from concourse import mybir
from concourse.bass_utils import run_bass_kernel_spmd

dt = mybir.dt
AF = mybir.ActivationFunctionType
ALU = mybir.AluOpType

H = W = 768
B_FULL = 16
N_CORES = 8
B_LOC = B_FULL // N_CORES  # 2 images per core

# tile geometry: 7 tiles; t<=5 start at row 124t-2, t=6 pinned so the invalid
# partition range starts at 96 (engine memset base must be 0/32/64/96)
TILES = []
for t in range(7):
    if t < 6:
        r0 = 124 * t - 2
        lo, hi = max(0, r0), r0 + 128
        out_r0, nv = 124 * t, 124
    else:
        r0 = 672
        lo, hi = 672, 768
        out_r0, nv = 744, 24
    p0, p1 = lo - r0, hi - r0
    ps = out_r0 - r0  # partition of first canonical output row
    TILES.append((r0, lo, hi, p0, p1, out_r0, nv, ps))

_PROG_CACHE = {}


def _band(taps, pscale=1.0):
    """128x128 band matrix M[pi, po] = taps[pi - po + 1] (correlation)."""
    m = np.zeros((128, 128), np.float32)
    for po in range(128):
        for d in (-1, 0, 1):
            pi = po + d
            if 0 <= pi < 128:
                m[pi, po] = taps[d + 1]
    return m * np.float32(pscale)


def _build_program(blur_ratio: float, loop_n: int = 0):
    """blur_ratio = bg/ag, the only input-derived compile-time constant.
    loop_n > 0 wraps the body in a device-side repeat loop (benchmarking)."""
    nc = bacc.Bacc("TRN2", target_bir_lowering=False, debug=False,
                   num_devices=N_CORES)

    IMG = nc.dram_tensor("img", [B_LOC, 3, H, W], dt.float32, kind="ExternalInput").ap()
    BBL = nc.dram_tensor("band_bl", [128, 128], dt.float32, kind="ExternalInput").ap()
    BGX = nc.dram_tensor("band_gx", [128, 128], dt.float32, kind="ExternalInput").ap()
    BGY = nc.dram_tensor("band_gy", [128, 128], dt.float32, kind="ExternalInput").ap()

    O_BL = nc.dram_tensor("blurred", [B_LOC, 3, H, W], dt.float32, kind="ExternalOutput").ap()
    O_GX = nc.dram_tensor("grad_x", [B_LOC, 1, H, W], dt.float32, kind="ExternalOutput").ap()
    O_GY = nc.dram_tensor("grad_y", [B_LOC, 1, H, W], dt.float32, kind="ExternalOutput").ap()
    O_GM = nc.dram_tensor("grad_mag", [B_LOC, 1, H, W], dt.float32, kind="ExternalOutput").ap()
    O_OR = nc.dram_tensor("grad_or", [B_LOC, 1, H, W], dt.float32, kind="ExternalOutput").ap()
    O_TH = nc.dram_tensor("thin", [B_LOC, 1, H, W], dt.float32, kind="ExternalOutput").ap()

    C1 = float(np.float32(360.0 / np.pi))   # 114.59156
    INV45 = float(np.float32(1.0 / 45.0))

    with tile.TileContext(nc) as tc, ExitStack() as ctx:
        cpool = ctx.enter_context(tc.tile_pool(name="const", bufs=1))
        halo = ctx.enter_context(tc.tile_pool(name="halo", bufs=1))
        imgp = ctx.enter_context(tc.tile_pool(name="img", bufs=2))
        work = ctx.enter_context(tc.tile_pool(name="work", bufs=2))
        blp = ctx.enter_context(tc.tile_pool(name="blp", bufs=2))
        psb = ctx.enter_context(tc.tile_pool(name="psb", bufs=2, space="PSUM"))
        psg = ctx.enter_context(tc.tile_pool(name="psg", bufs=1, space="PSUM"))

        band_bl = cpool.tile([128, 128], dt.float32, tag="band_bl")
        band_gx = cpool.tile([128, 128], dt.float32, tag="band_gx")
        band_gy = cpool.tile([128, 128], dt.float32, tag="band_gy")
        nc.sync.dma_start(band_bl[:], BBL[:])
        nc.sync.dma_start(band_gx[:], BGX[:])
        nc.sync.dma_start(band_gy[:], BGY[:])

        # persistent halo'd buffers (border cols stay zero forever), x2 parity
        tcb = [[halo.tile([128, W + 2], dt.float32, tag=f"tc{p}{c}", name=f"tc{p}{c}")
                for c in range(3)] for p in range(2)]
        gb = [halo.tile([128, W + 2], dt.float32, tag=f"g{p}", name=f"gbuf{p}") for p in range(2)]
        hb = [halo.tile([128, W + 2], dt.float32, tag=f"h{p}", name=f"hbuf{p}") for p in range(2)]
        for p in range(2):
            for c in range(3):
                nc.vector.memset(tcb[p][c][:], 0.0)
            nc.vector.memset(gb[p][:], 0.0)
            nc.vector.memset(hb[p][:], 0.0)

        loop_ctx = tc.For_i(0, loop_n, 1) if loop_n > 0 else None
        if loop_ctx is not None:
            ctx.enter_context(loop_ctx)

        act_chain = []  # forced order of set-switching ACT ops (sqrt/atan batches)
        for b in range(B_LOC):
            s_tiles, r_tiles = [], []
            # ---- phase 1: convs, gx/gy, squares, ratio (per tile) ----
            for t in range(7):
                r0, lo, hi, p0, p1, out_r0, nv, ps = TILES[t]
                par = (b * 7 + t) % 2

                ims = []
                for c in range(3):
                    im = imgp.tile([128, W], dt.float32, tag=f"img{c}", name=f"im{b}_{t}_{c}")
                    if p0 > 0:
                        nc.vector.memset(im[0:p0, :], 0.0)
                    if p1 < 128:
                        nc.vector.memset(im[p1:128, :], 0.0)
                    nc.sync.dma_start(im[p0:p1, :], IMG[b, c, lo:hi, :])
                    ims.append(im)

                # ---- gaussian blur ----
                bls = []
                for c in range(3):
                    pb = psb.tile([128, W], dt.float32, tag="psB", name=f"psB{b}_{t}_{c}")
                    nc.tensor.matmul(pb[:, 0:512], band_bl[:], ims[c][:, 0:512],
                                     start=True, stop=True)
                    nc.tensor.matmul(pb[:, 512:768], band_bl[:], ims[c][:, 512:768],
                                     start=True, stop=True)
                    tcx = tcb[par][c]
                    nc.scalar.copy(tcx[:, 1:769], pb[:, 0:768])
                    u = work.tile([128, W], dt.float32, tag="u", name=f"u{b}_{t}_{c}")
                    nc.gpsimd.tensor_tensor(u[:], tcx[:, 0:768], tcx[:, 2:770], ALU.add)
                    bl = blp.tile([128, W], dt.float32, tag=f"bl{c}", name=f"bl{b}_{t}_{c}")
                    nc.vector.scalar_tensor_tensor(
                        bl[:], tcx[:, 1:769], blur_ratio, u[:], ALU.mult, ALU.add)
                    nc.sync.dma_start(O_BL[b, c, out_r0:out_r0 + nv, :],
                                      bl[ps:ps + nv, :])
                    bls.append(bl)

                m01 = work.tile([128, W], dt.float32, tag="m01", name=f"m01_{b}_{t}")
                nc.gpsimd.tensor_tensor(m01[:], bls[0][:], bls[1][:], ALU.add)
                mb = work.tile([128, W], dt.float32, tag="mb", name=f"mb{b}_{t}")
                nc.vector.tensor_tensor(mb[:], bls[2][:], m01[:], ALU.add)
                if t == 0:
                    nc.vector.memset(mb[0:2, :], 0.0)   # row -1 pad for sobel
                if t == 6:
                    nc.vector.memset(mb[96:128, :], 0.0)  # row 768 pad

                # ---- sobel ----
                pg = psg.tile([128, W], dt.float32, tag="psG", name=f"psG{b}_{t}")
                nc.tensor.matmul(pg[:, 0:512], band_gx[:], mb[:, 0:512],
                                 start=True, stop=True)
                nc.tensor.matmul(pg[:, 512:768], band_gx[:], mb[:, 512:768],
                                 start=True, stop=True)
                g = gb[par]
                nc.scalar.copy(g[:, 1:769], pg[:, 0:768])

                ph = psg.tile([128, W], dt.float32, tag="psH", name=f"psH{b}_{t}")
                nc.tensor.matmul(ph[:, 0:512], band_gy[:], mb[:, 0:512],
                                 start=True, stop=True)
                nc.tensor.matmul(ph[:, 512:768], band_gy[:], mb[:, 512:768],
                                 start=True, stop=True)
                hh = hb[par]
                nc.scalar.copy(hh[:, 1:769], ph[:, 0:768])

                gx = work.tile([128, W], dt.float32, tag="gx", bufs=8, name=f"gx{b}_{t}")
                nc.vector.tensor_tensor(gx[:], g[:, 2:770], g[:, 0:768], ALU.subtract)
                s1 = work.tile([128, W + 1], dt.float32, tag="s1", name=f"s1_{b}_{t}")
                nc.vector.tensor_tensor(s1[:], hh[:, 0:769], hh[:, 1:770], ALU.add)
                gy = work.tile([128, W], dt.float32, tag="gy", bufs=8, name=f"gy{b}_{t}")
                nc.vector.tensor_tensor(gy[:], s1[:, 0:768], s1[:, 1:769], ALU.add)

                nc.sync.dma_start(O_GX[b, 0, out_r0:out_r0 + nv, :], gx[ps:ps + nv, :])
                nc.sync.dma_start(O_GY[b, 0, out_r0:out_r0 + nv, :], gy[ps:ps + nv, :])
                s_tiles.append(gx)
                r_tiles.append(gy)

            # ---- phase 2: squares + all sqrts (single ACT table set) ----
            for t in range(7):
                _, _, _, _, _, out_r0, nv, ps = TILES[t]
                gx, gy = s_tiles[t], r_tiles[t]
                sq1 = work.tile([128, W], dt.float32, tag="sq1", bufs=1, name=f"sq1_{b}_{t}")
                nc.scalar.activation(sq1[:], gx[:], AF.Square)
                sq2 = work.tile([128, W], dt.float32, tag="sq2", bufs=1, name=f"sq2_{b}_{t}")
                nc.scalar.activation(sq2[:], gy[:], AF.Square)
                s = work.tile([128, W], dt.float32, tag="s", name=f"s{b}_{t}")
                nc.gpsimd.tensor_tensor(s[:], sq1[:], sq2[:], ALU.add)
                gm = work.tile([128, W], dt.float32, tag="gm", bufs=4, name=f"gm{b}_{t}")
                act_chain.append(nc.scalar.activation(gm[:], s[:], AF.Sqrt))
                nc.sync.dma_start(O_GM[b, 0, out_r0:out_r0 + nv, :], gm[ps:ps + nv, :])
                nc.sync.dma_start(O_TH[b, 0, out_r0:out_r0 + nv, :], gm[ps:ps + nv, :])

            # ---- phase 3: division + all arctans (single ACT table set) ----
            for t in range(7):
                _, _, _, _, _, out_r0, nv, ps = TILES[t]
                gx, gy = s_tiles[t], r_tiles[t]
                rec = work.tile([128, W], dt.float32, tag="rec", name=f"rec{b}_{t}")
                rscr = work.tile([128, W], dt.float32, tag="rscr", bufs=1, name=f"rscr{b}_{t}")
                nc.vector.reciprocal_approx_accurate(out=rec[:], in_=gx[:], scratch=rscr[:])
                r = work.tile([128, W], dt.float32, tag="r", name=f"r{b}_{t}")
                nc.vector.tensor_tensor(r[:], rec[:], gy[:], ALU.mult)
                at = work.tile([128, W], dt.float32, tag="at", bufs=4, name=f"at{b}_{t}")
                act_chain.append(nc.scalar.activation(at[:], r[:], AF.Arctan))
                z = work.tile([128, W], dt.float32, tag="z", bufs=1, name=f"z{b}_{t}")
                nc.vector.tensor_scalar(z[:], at[:], C1, 180.0, ALU.mult, ALU.add)
                oi = work.tile([128, W], dt.int32, tag="oi", name=f"oi{b}_{t}")
                nc.vector.tensor_scalar(oi[:], z[:], INV45, None, ALU.mult)
                ori = work.tile([128, W], dt.float32, tag="ori", name=f"ori{b}_{t}")
                nc.vector.tensor_scalar(ori[:], oi[:], 45.0, None, ALU.mult)
                nc.sync.dma_start(O_OR[b, 0, out_r0:out_r0 + nv, :], ori[ps:ps + nv, :])

        # force sqrt-batch / arctan-batch ordering on ScalarE so the LUT
        # table set switches only at batch boundaries
        for i in range(1, len(act_chain)):
            bass._add_dep_helper(act_chain[i].ins, act_chain[i - 1].ins,
                                 sync=False, reason="act-set-batch")

    nc.compile()
    return nc


def _prep(w_gauss, w_sobel_x, w_sobel_y):
    f32 = np.float32
    g2 = np.asarray(w_gauss, np.float64)[0, 0]
    g1 = np.sqrt(np.diag(g2))
    assert np.allclose(np.outer(g1, g1), g2, rtol=1e-5), "gaussian not separable"
    ag, bg = f32(g1[0]), f32(g1[1])
    assert abs(g1[0] - g1[2]) < 1e-12
    band_bl = _band([f32(ag * ag), f32(ag * bg), f32(ag * ag)])

    sx = np.asarray(w_sobel_x, np.float64)[0, 0]
    vx, hx = sx[:, 2], sx[1, :]
    assert np.allclose(np.outer(vx, hx), sx, atol=1e-7), "sobel_x not separable"
    assert hx[1] == 0.0 and abs(hx[0] + hx[2]) < 1e-12
    band_gx = _band((vx * hx[2] / 3.0).astype(f32))

    sy = np.asarray(w_sobel_y, np.float64)[0, 0]
    vy, hy = sy[:, 1], sy[2, :]
    assert np.allclose(np.outer(vy, hy), sy, atol=1e-7), "sobel_y not separable"
    assert abs(hy[0] - hy[2]) < 1e-12 and abs(hy[1] - 2.0 * hy[0]) < 1e-12
    band_gy = _band((vy * hy[0] / 3.0).astype(f32))

    blur_ratio = float(f32(bg) / f32(ag))
    return band_bl, band_gx, band_gy, blur_ratio


def kernel(img, w_gauss, w_sobel_x, w_sobel_y, w_dir):
    img = np.ascontiguousarray(np.asarray(img, np.float32))
    assert img.shape == (B_FULL, 3, H, W)
    wd = np.asarray(w_dir, np.float32)
    ident = np.zeros((8, 1, 3, 3), np.float32)
    ident[:, 0, 1, 1] = 1.0
    assert np.array_equal(wd, ident), "w_dir not identity kernels"

    band_bl, band_gx, band_gy, blur_ratio = _prep(w_gauss, w_sobel_x, w_sobel_y)

    key = (blur_ratio, band_bl.tobytes(), band_gx.tobytes(), band_gy.tobytes())
    if key not in _PROG_CACHE:
        _PROG_CACHE.clear()
        _PROG_CACHE[key] = _build_program(blur_ratio)
    nc = _PROG_CACHE[key]

    in_maps = []
    for c in range(N_CORES):
        in_maps.append({
            "img": img[B_LOC * c:B_LOC * (c + 1)],
            "band_bl": band_bl, "band_gx": band_gx, "band_gy": band_gy,
        })
    res = run_bass_kernel_spmd(nc, in_maps, core_ids=list(range(N_CORES)))

    def cat(name):
        return np.concatenate([res.results[c][name] for c in range(N_CORES)], axis=0)

    blurred = cat("blurred")
    grad_x = cat("grad_x")
    grad_y = cat("grad_y")
    grad_mag = cat("grad_mag")
    grad_or = cat("grad_or")
    thin = cat("thin")
    return blurred, grad_x, grad_y, grad_mag, grad_or, thin
